# revision 38
# baseline (speedup 1.0000x reference)
"""Chamfer distance (adv->ori direction) Trainium2 Bass kernel.

Problem: adv_pc [8, 4096, 3], ori_pc [8, 4096, 3], weights [8] ->
scalar f32 loss = mean_b( w_b * mean_k( min_j ||adv_bk - ori_bj||^2 ) ).

Sharding: data parallel over the batch dim — core b handles batch b.

Per-core algorithm (K = 4096 points):
  m'[k, j]  = b2_j/2 - a_k . b_j        (augmented matmul, contract dim 4:
                                         ahat = (-a, 1), bhat = (b, b2/2))
  out_core  = sum_k ( a2_k + 2 * min_j m'[k, j] )     (= 4096 * loss1_b)
a2_k is added per-point BEFORE the sum over k (the min is ~ -1.5 and a2
~ +3.0; summing them separately would lose the ~0.002 result to
cancellation).

PE work is done in a 3-term bf16 decomposition of the fp32 operands
(x = xh + xl exactly, both bf16): m' = ah.bh + ah.bl + al.bh, dropping
al.bl (~1e-5, zero-mean — vanishes in the 4096-point mean). Three bf16
passes/matmul beat the hardware's two half-rate fp32 passes and halve
LDWEIGHTS cost. The ones-row of ahat is exact in bf16, so b2/2's
residual is carried in bl's 4th row.

Each [4, *] operand is replicated into PE row groups 0/32/64/96 so four
matmuls (one per PSUM bank) run concurrently via tile_position.

The j-min over each wave's 4 banks [128, 2048]: ScalarE copies banks
2-3 to SBUF while VectorE runs a fused tensor_tensor_reduce
min(banks 0-1, copy) -> per-point min, reading 2 elems/cycle.
"""

import numpy as np

B = 8
K = 4096
KT = K // 128  # 32 k-tiles of 128 adv points
NW = 2 * KT    # 64 waves of 2048 j each
NCORES = 8

_NC_CACHE = {}


def _build_nc():
    import concourse.bacc as bacc
    import concourse.mybir as mybir
    import concourse.tile as tile
    from concourse import masks

    f32 = mybir.dt.float32
    bf16 = mybir.dt.bfloat16
    Alu = mybir.AluOpType
    Ax = mybir.AxisListType

    nc = bacc.Bacc("TRN2", target_bir_lowering=False, debug=False,
                   num_devices=NCORES)

    adv = nc.dram_tensor("adv", [K, 3], f32, kind="ExternalInput").ap()
    ori = nc.dram_tensor("ori", [K, 3], f32, kind="ExternalInput").ap()
    out = nc.dram_tensor("out", [1, 1], f32, kind="ExternalOutput").ap()

    with tile.TileContext(nc) as tc:
        with tc.tile_pool(name="consts", bufs=1) as consts, \
             tc.tile_pool(name="sb", bufs=1) as sb:
            ident = consts.tile([128, 128], f32)
            masks.make_identity(nc, ident[:])

            # One fully contiguous DMA per tensor into point-major landing
            # tiles (partition p = points 32p..32p+31 as xyz triples),
            # then a strided DVE copy into coord-block staging [128, 128]:
            # col 32q+n, q=0 -> 4th coord (ones / b2/2), q=1..3 -> coords.
            # Point identity: (p, n) = input point 32p+n — a permutation
            # of the input order, identical for both tensors, and min/mean
            # are permutation-invariant.
            Pa = sb.tile([128, 3 * KT], f32)
            Po = sb.tile([128, 3 * KT], f32)
            Av = sb.tile([128, 4 * KT], f32)
            Ov = sb.tile([128, 4 * KT], f32)
            ones_t = consts.tile([128, 1], f32)
            nc.gpsimd.memset(ones_t[:], 1.0)
            # -1.0: the whole adv side is scaled by -1 during the bf16
            # split, which turns this 4th coord back into +1.
            nc.gpsimd.memset(Av[:, 0:KT], -1.0)
            nc.sync.dma_start(
                out=Pa[:], in_=adv.rearrange("(p c) d -> p (c d)", p=128))
            nc.scalar.dma_start(
                out=Po[:], in_=ori.rearrange("(p c) d -> p (c d)", p=128))
            Pa_dmaj = Pa[:].rearrange("p (n d) -> p d n", d=3)
            Po_dmaj = Po[:].rearrange("p (n d) -> p d n", d=3)
            Av_cb = Av[:, KT:].rearrange("p (d n) -> p d n", d=3)
            Ov_cb = Ov[:, KT:].rearrange("p (d n) -> p d n", d=3)
            nc.vector.tensor_copy(Av_cb, Pa_dmaj)
            nc.vector.tensor_copy(Ov_cb, Po_dmaj)

            # a2 per adv point -> a2arr [128, 32] (a2arr[p, n] = point
            # 32p+n); b2/2 per ori point -> col n of Ov.
            Asq = sb.tile([128, 3 * KT], f32)
            Osq = sb.tile([128, 3 * KT], f32)
            a2arr = sb.tile([128, KT], f32)
            nc.vector.tensor_tensor(Asq[:], Pa[:], Pa[:], op=Alu.mult)
            nc.vector.tensor_tensor(Osq[:], Po[:], Po[:], op=Alu.mult)
            Asq_v = Asq[:].rearrange("p (n d) -> p n d", d=3)
            Osq_v = Osq[:].rearrange("p (n d) -> p n d", d=3)
            nc.vector.tensor_reduce(a2arr[:], Asq_v, axis=Ax.X, op=Alu.add)
            nc.vector.tensor_reduce(Ov[:, 0:KT], Osq_v, axis=Ax.X,
                                    op=Alu.add)
            nc.vector.tensor_scalar_mul(Ov[:, 0:KT], Ov[:, 0:KT], 0.5)

            # One PE transpose per tensor -> PSUM [128, 128] (row 32q+t =
            # coord q of k-tile t); copy to SBUF, bf16 hi/lo split, then
            # DMA-gather rows into the operand layout replicated to PE row
            # groups 0/32/64/96 for 4-way matmul concurrency. HLa/HLo rows
            # 32g+(0..3), col t*256 + hl*128 + p: per-k-tile hi|lo pair.
            HLa = sb.tile([128, 2 * K], bf16)
            HLo = sb.tile([128, 2 * K], bf16)
            Sa = sb.tile([128, 128], f32)
            So = sb.tile([128, 128], f32)
            SBa = sb.tile([128, 256], bf16)
            SBo = sb.tile([128, 256], bf16)
            # Stage-interleaved so the two tensors' chains overlap: both
            # transposes, both copies, both splits, then gathers/replicas
            # with a dedicated DMA trigger engine per tensor.
            tens = ((Ov, So, SBo, HLo, 1.0, nc.sync),
                    (Av, Sa, SBa, HLa, -1.0, nc.scalar))
            with tc.tile_pool(name="tp", bufs=2, space="PSUM") as tp:
                tpts = []
                for src, S, SB, HL, sgn, eng in tens:
                    tpt = tp.tile([128, 128], f32, tag="tpt")
                    # rows 32q+n: q=0 = 4th coord (ones / b2/2), q=1..3 =
                    # coords of point 32p+n (contract-row order arbitrary).
                    nc.tensor.transpose(tpt[:], src[:], ident[:])
                    tpts.append(tpt)
                for (src, S, SB, HL, sgn, eng), tpt in zip(tens, tpts):
                    nc.vector.tensor_copy(S[:], tpt[:])
                for src, S, SB, HL, sgn, eng in tens:
                    # exact split: sgn*S = hi + lo with hi = bf16(sgn*S);
                    # SB col hl*128 + p. The adv sign (-1) rides here so
                    # the matmul computes b2/2 - a.b (its 4th coord was
                    # staged as -1, flipping back to +1).
                    nc.vector.tensor_scalar_mul(SB[:, 0:128], S[:], sgn)
                    nc.vector.scalar_tensor_tensor(
                        out=SB[:, 128:256], in0=S[:], scalar=sgn,
                        in1=SB[:, 0:128], op0=Alu.mult, op1=Alu.subtract)
                # SB element (32q+n, hl*128+p) -> HL row 32g+q,
                # col n*256 + hl*128 + p; per-q gathers into row group 0,
                # then row-group replicas.
                for r in (0, 32, 64, 96):
                    for src, S, SB, HL, sgn, eng in tens:
                        dst_v = HL[r:r + 4, :].rearrange(
                            "q (t c) -> q t c", c=256)
                        eng.dma_start(out=dst_v[:], in_=SB[:])

            # Main loop: per wave, 4 PSUM banks [128, 512] are filled by
            # 4-way concurrent 3-pass bf16 matmuls; ScalarE copies banks
            # 2-3 to SBUF, VectorE fuses min(banks 0-1, copy) + j-reduce.
            gminP = sb.tile([128, NW], f32)
            with tc.tile_pool(name="mm", bufs=2, space="PSUM") as mm, \
                 tc.tile_pool(name="cp", bufs=2) as cp, \
                 tc.tile_pool(name="sc", bufs=2) as scp:
                for w in range(NW):
                    t, h = divmod(w, 2)
                    ps = mm.tile([128, 2048], f32, tag="ps")
                    for g in range(4):
                        jt = (h * 4 + g) * 4  # first of 4 j k-tiles
                        r = 32 * g
                        a_hi = HLa[r:r + 4, t * 256:t * 256 + 128]
                        a_lo = HLa[r:r + 4, t * 256 + 128:(t + 1) * 256]
                        bv = HLo[r:r + 4, :].rearrange(
                            "q (t hl p) -> q t hl p", hl=2, p=128)
                        b_hi = bv[:, jt:jt + 4, 0, :]
                        b_lo = bv[:, jt:jt + 4, 1, :]
                        o = ps[:, g * 512:(g + 1) * 512]
                        nc.tensor.matmul(o, a_hi, b_hi, start=True,
                                         stop=False, tile_position=(r, 0))
                        nc.tensor.matmul(o, a_hi, b_lo, start=False,
                                         stop=False, tile_position=(r, 0))
                        nc.tensor.matmul(o, a_lo, b_hi, start=False,
                                         stop=True, tile_position=(r, 0))
                    nc.vector.tensor_reduce(
                        gminP[:, w:w + 1], ps[:], axis=Ax.X, op=Alu.min)

                # Combine: min over the two waves per k-tile, then
                # 2*min + a2 per point, sum over points, partition-sum.
                gmin2 = sb.tile([128, KT], f32)
                tot = sb.tile([128, KT], f32)
                ksum = sb.tile([128, 1], f32)
                res = sb.tile([1, 1], f32)
                gminP_v = gminP[:].rearrange("p (t h) -> p t h", h=2)
                nc.vector.tensor_reduce(gmin2[:], gminP_v, axis=Ax.X,
                                        op=Alu.min)
                nc.vector.scalar_tensor_tensor(
                    out=tot[:], in0=gmin2[:], scalar=2.0, in1=a2arr[:],
                    op0=Alu.mult, op1=Alu.add)
                nc.vector.tensor_reduce(ksum[:], tot[:], axis=Ax.X,
                                        op=Alu.add)
                ps = mm.tile([128, 2048], f32, tag="ps")
                nc.tensor.matmul(ps[:1, :1], ksum[:], ones_t[:],
                                 start=True, stop=True)
                nc.vector.tensor_copy(res[:], ps[:1, :1])
                nc.sync.dma_start(out=out[:], in_=res[:])

    nc.compile()
    return nc


def _get_nc():
    if "nc" not in _NC_CACHE:
        _NC_CACHE["nc"] = _build_nc()
    return _NC_CACHE["nc"]


def kernel(adv_pc, ori_pc, weights):
    from concourse.bass_utils import run_bass_kernel_spmd

    adv_pc = np.asarray(adv_pc, dtype=np.float32)
    ori_pc = np.asarray(ori_pc, dtype=np.float32)
    weights = np.asarray(weights, dtype=np.float32)

    nc = _get_nc()
    in_maps = [
        {"adv": np.ascontiguousarray(adv_pc[b]),
         "ori": np.ascontiguousarray(ori_pc[b])}
        for b in range(B)
    ]
    res = run_bass_kernel_spmd(nc, in_maps, core_ids=list(range(NCORES)))
    sums = np.array([res.results[b]["out"][0, 0] for b in range(B)],
                    dtype=np.float32)
    loss1 = sums / np.float32(K)
    return np.array(np.mean(loss1 * weights), dtype=np.float32)


if __name__ == "__main__":
    rng = np.random.default_rng(0)
    a = rng.standard_normal((B, K, 3), dtype=np.float32)
    o = rng.standard_normal((B, K, 3), dtype=np.float32)
    w = np.ones((B,), dtype=np.float32)
    print(kernel(a, o, w))
